# revision 32
# baseline (speedup 1.0000x reference)
"""Trainium2 Bass kernel: 2-layer LSTM (B=1024, T=512, H=256) + linear head.

Data-parallel across 8 NeuronCores: each core runs the full sequential scan
for a 128-row batch shard. Host-side work is marshaling only: sharding,
weight transposes/permutation, folding the day-embedding into layer-0 input
weights, one-hot encoding the day column, and fp8 packing.

All matmuls/states bf16 (the HW runs fp8 DoubleRow at the same 1 col/cycle
rate, so bf16 costs nothing and keeps rel-err ~6e-3). Per timestep: 16 N=512
matmuls + 4 transposes on PE, 8 sigmoid/tanh on ACT, merged [i*g | f*c] +
cell/hidden ops on DVE (bf16 SBUF fast modes). Emission order is tuned for
the greedy list scheduler: stale-dependency matmuls (aug/bias/hh1) fill the
h0T chain latency; each layer's full cell is emitted before the other
layer's sigmoid block; L0's transpose+cast is decoupled from L1's cell tail.
Gate columns permuted [i f o g] so one sigmoid covers i,f.
"""

import sys

import numpy as np

try:
    import concourse.bass as _probe  # noqa: F401
except ImportError:
    sys.path.insert(0, "/opt/trn_rl_repo")

import ml_dtypes

F8 = ml_dtypes.float8_e4m3
BF = ml_dtypes.bfloat16

B_FULL, T, D, H, P_OUT = 1024, 512, 64, 256, 14
N_CORES = 8
B = B_FULL // N_CORES  # 128 rows per core
G = 4 * H  # 1024 gate width
FA = 16  # augmented input rows: [val, onehot(day) x7, ones, pad x7]
CH = 64  # timesteps per aug SBUF chunk
NCH = T // CH

_PERM = np.concatenate(
    [np.arange(0, 512), np.arange(768, 1024), np.arange(512, 768)]
)  # [i f o g]

_MODULE = None
LAST_RESULTS = None
SCHED_P = 0.0  # pacing period ns (0 = disabled)
SCHED_D = 0.0  # L1-sigmoid phase offset ns


def _build_module():
    from contextlib import ExitStack

    import concourse.mybir as mybir
    from concourse import bacc
    from concourse.masks import make_identity
    from concourse.tile import TileContext

    f32 = mybir.dt.float32
    f32r = mybir.dt.float32r
    bf16 = mybir.dt.bfloat16
    fp8 = mybir.dt.float8e4
    Sig = mybir.ActivationFunctionType.Sigmoid
    Tanh = mybir.ActivationFunctionType.Tanh
    DR = mybir.MatmulPerfMode.DoubleRow

    nc = bacc.Bacc()
    aug_d = nc.dram_tensor("aug", [FA, T * B], bf16, kind="ExternalInput")
    w0t_d = nc.dram_tensor("w0t", [128, G], bf16, kind="ExternalInput")
    whh0t_d = nc.dram_tensor("whh0t", [128, 2 * G], bf16, kind="ExternalInput")
    wih1t_d = nc.dram_tensor("wih1t", [128, 2 * G], bf16, kind="ExternalInput")
    whh1t_d = nc.dram_tensor("whh1t", [128, 2 * G], bf16, kind="ExternalInput")
    e0p_d = nc.dram_tensor("e0p", [128, 128], bf16, kind="ExternalInput")
    b1f_d = nc.dram_tensor("b1f", [128, G], bf16, kind="ExternalInput")
    e032_d = nc.dram_tensor("e032", [128, 128], f32r, kind="ExternalInput")
    wlint_d = nc.dram_tensor("wlint", [H, P_OUT], f32r, kind="ExternalInput")
    blinf_d = nc.dram_tensor("blinf", [128, P_OUT], f32r, kind="ExternalInput")
    out_d = nc.dram_tensor("out", [B, P_OUT], f32, kind="ExternalOutput")

    with TileContext(nc) as tc, ExitStack() as ctx:
        consts = ctx.enter_context(tc.tile_pool(name="consts", bufs=1))
        h0Tp = ctx.enter_context(tc.tile_pool(name="h0Tp", bufs=3))
        h1Tp = ctx.enter_context(tc.tile_pool(name="h1Tp", bufs=3))
        gc0p = ctx.enter_context(tc.tile_pool(name="gc0p", bufs=3))
        gc1p = ctx.enter_context(tc.tile_pool(name="gc1p", bufs=3))
        acts = ctx.enter_context(tc.tile_pool(name="acts", bufs=2))
        g0pp = ctx.enter_context(tc.tile_pool(name="g0pp", bufs=1, space="PSUM"))
        g1pp = ctx.enter_context(tc.tile_pool(name="g1pp", bufs=2, space="PSUM"))
        hTps = ctx.enter_context(tc.tile_pool(name="hTps", bufs=2, space="PSUM"))

        # --- constants to SBUF (fp8 weights, k-tile pairs interleaved) ---
        def load_w(dram, tag):
            sb = consts.tile([128, 2 * G], bf16, tag=tag)
            nc.sync.dma_start(sb, dram[:, :])
            return sb

        w0t_sb = consts.tile([128, G], bf16, tag="w0t")
        nc.sync.dma_start(w0t_sb, w0t_d[:, :])
        whh0t_sb = load_w(whh0t_d, "whh0t")
        wih1t_sb = load_w(wih1t_d, "wih1t")
        whh1t_sb = load_w(whh1t_d, "whh1t")
        b1f_sb = consts.tile([128, G], bf16, tag="b1f")
        nc.sync.dma_start(b1f_sb, b1f_d[:, :])
        e0p_sb = consts.tile([128, 128], bf16, tag="e0p")
        nc.sync.dma_start(e0p_sb, e0p_d[:, :])
        e032_sb = consts.tile([128, 128], f32r, tag="e032")
        nc.sync.dma_start(e032_sb, e032_d[:, :])
        wlint_sb = consts.tile([128, 2 * P_OUT], f32r, tag="wlint")
        for k in range(2):
            nc.sync.dma_start(
                wlint_sb[:, k * P_OUT : (k + 1) * P_OUT],
                wlint_d[k * 128 : (k + 1) * 128, :],
            )
        blinf_sb = consts.tile([128, P_OUT], f32r, tag="blinf")
        nc.sync.dma_start(blinf_sb, blinf_d[:, :])
        ident = consts.tile([128, 128], bf16, tag="ident")
        make_identity(nc, ident)

        def kslice(sb, k, nb):
            # moving AP [128, 512]: k-tile k, psum bank nb
            return sb[:, k * G + nb * 512 : k * G + (nb + 1) * 512]


        # Two persistent aug buffers; one extra B-column pad so the DR pair
        # at t%CH==CH-1 can read a (zero-weighted) second k-tile.
        aug_bufs = [
            consts.tile([128, (CH + 1) * B], bf16, tag=f"augbuf{i}", name=f"augbuf{i}")
            for i in range(2)
        ]

        def load_chunk(chi):
            nc.sync.dma_start(
                aug_bufs[chi % 2][0:FA, 0 : CH * B],
                aug_d[:, chi * CH * B : (chi + 1) * CH * B],
            )

        for ab in aug_bufs:
            nc.gpsimd.memset(ab, 0.0)
        load_chunk(0)
        load_chunk(1)

        mm = nc.tensor.matmul

        h0T = [None] * T
        h1T = [None] * T
        h0n = [None] * T
        h1n = [None] * T
        sig = [[None] * T, [None] * T]
        gc = [[None] * (T + 1), [None] * (T + 1)]
        g0ps = [None] * T
        g1ps = [None] * T
        h1tps = [None] * T
        bk = [slice(0, 512), slice(512, 1024)]

        def aug_st(t):
            chi = t // CH
            s = (t % CH) * B
            return aug_bufs[chi % 2][:, s : s + B]

        def emit_g0_mms(t):
            chi = t // CH
            if t % CH == 0 and chi + 2 < NCH:
                load_chunk(chi + 2)
            ap = aug_st(t)
            g0 = g0pp.tile([B, G], f32, tag="g0", name=f"g0_{t}")
            g0ps[t] = g0
            for nb in range(2):
                mm(g0[:, bk[nb]], ap, w0t_sb[:, bk[nb]],
                   start=True, stop=(t == 0))
            if t == 0:
                return

        def emit_g0_hh0(t):
            if t == 0:
                return
            g0 = g0ps[t]
            hs = h0T[t - 1]
            for nb in range(2):
                for k in range(2):
                    mm(g0[:, bk[nb]], hs[:, k * 128 : (k + 1) * 128],
                       kslice(whh0t_sb, k, nb), start=False, stop=(k == 1))

        def emit_g1_bias(t):
            g1 = g1pp.tile([B, G], f32, tag="g1", name=f"g1_{t}")
            g1ps[t] = g1
            for nb in range(2):
                mm(g1[:, bk[nb]], e0p_sb, b1f_sb[:, bk[nb]],
                   start=True, stop=False)

        def emit_g1_hh1(t):
            if t == 0:
                return
            g1 = g1ps[t]
            hq = h1T[t - 1]
            for nb in range(1, -1, -1):
                for k in range(2):
                    mm(g1[:, bk[nb]], hq[:, k * 128 : (k + 1) * 128],
                       kslice(whh1t_sb, k, nb), start=False, stop=(k == 1))

        def emit_g1_ih1(t):
            g1 = g1ps[t]
            hp = h0T[t]
            last = t == 0  # no hh1 at t=0, so ih1 closes the group
            for nb in range(2):
                for k in range(2):
                    mm(g1[:, bk[nb]], hp[:, k * 128 : (k + 1) * 128],
                       kslice(wih1t_sb, k, nb), start=False, stop=(last and k == 1))

        def emit_sig(layer, t):
            gps = g0ps[t] if layer == 0 else g1ps[t]
            s = acts.tile([B, 3 * H], bf16, tag=f"sigifo{layer}", name=f"sigifo{layer}_{t}")
            sig[layer][t] = s
            nc.scalar.activation(s[:, 0 : 2 * H], gps[:, 0 : 2 * H], Sig)
            nc.scalar.activation(gc[layer][t][:, 0:H], gps[:, 3 * H : G], Tanh)
            nc.scalar.activation(s[:, 2 * H : 3 * H], gps[:, 2 * H : 3 * H], Sig)
            return s[:, 2 * H : 3 * H]

        def emit_cell_dve(layer, t):
            gcp = gc0p if layer == 0 else gc1p
            s = sig[layer][t][:, 0 : 2 * H]
            g_c = gc[layer][t]
            igfc = acts.tile([B, 2 * H], bf16, tag=f"igfc{layer}", name=f"igfc{layer}_{t}")
            nc.vector.tensor_mul(igfc, s, g_c)
            nxt = gcp.tile([B, 2 * H], bf16, tag=f"gc{layer}", name=f"gc{layer}_{t + 1}")
            gc[layer][t + 1] = nxt
            nc.vector.tensor_add(nxt[:, H : 2 * H], igfc[:, 0:H], igfc[:, H : 2 * H])

        def emit_tanh_c(layer, t):
            tcx = acts.tile([B, H], bf16, tag=f"tc{layer}", name=f"tc{layer}_{t}")
            nc.scalar.activation(tcx, gc[layer][t + 1][:, H : 2 * H], Tanh)
            return tcx

        def emit_hmul(layer, t, so, tcx):
            h = acts.tile([B, H], bf16, tag=f"hn{layer}", name=f"hn{layer}_{t}")
            nc.vector.tensor_mul(h, so, tcx)
            if layer == 0:
                h0n[t] = h
            else:
                h1n[t] = h

        def emit_h0_transp(t):
            ps = hTps.tile([128, 256], bf16, tag="htp", name=f"h0tp_{t}")
            nc.tensor.transpose(ps[:, 0:128], h0n[t][:, 0:128], ident)
            nc.tensor.transpose(ps[:, 128:256], h0n[t][:, 128:256], ident)
            hsb = h0Tp.tile([128, H], bf16, tag="h0T", name=f"h0T_{t}")
            nc.vector.tensor_copy(hsb, ps)
            h0T[t] = hsb

        def emit_h1_transp(t):
            ps = hTps.tile([128, 256], bf16, tag="htp", name=f"h1tp_{t}")
            nc.tensor.transpose(ps[:, 0:128], h1n[t][:, 0:128], ident)
            nc.tensor.transpose(ps[:, 128:256], h1n[t][:, 128:256], ident)
            h1tps[t] = ps

        def emit_h1_cast(t):
            hsb = h1Tp.tile([128, H], bf16, tag="h1T", name=f"h1T_{t}")
            nc.vector.tensor_copy(hsb, h1tps[t])
            h1T[t] = hsb

        # initial gc tiles (c-half zeroed; i*0 contributes nothing at t=0)
        for layer, gcp in ((0, gc0p), (1, gc1p)):
            t0 = gcp.tile([B, 2 * H], bf16, tag=f"gc{layer}", name=f"gc{layer}_0")
            gc[layer][0] = t0
            nc.vector.memset(t0[:, H : 2 * H], 0.0)

        for tau in range(T + 2):
            t0, t1, t2 = tau, tau - 1, tau - 2
            if t0 < T:
                emit_g0_mms(t0)
            if 0 <= t2 < T:
                emit_h1_transp(t2)
                emit_h1_cast(t2)
            if t0 < T:
                emit_g0_hh0(t0)
            if 0 <= t1 < T:
                emit_g1_bias(t1)
                emit_g1_ih1(t1)
            if t0 < T:
                so0 = emit_sig(0, t0)
                emit_cell_dve(0, t0)
                tc0 = emit_tanh_c(0, t0)
                emit_hmul(0, t0, so0, tc0)
                emit_h0_transp(t0)
            if 0 <= t1 < T:
                emit_g1_hh1(t1)
                if SCHED_P > 0:
                    with tc.tile_wait_until((tau * SCHED_P + SCHED_D) * 1e-6):
                        so1 = emit_sig(1, t1)
                else:
                    so1 = emit_sig(1, t1)
                emit_cell_dve(1, t1)
                tc1 = emit_tanh_c(1, t1)
                emit_hmul(1, t1, so1, tc1)

        # ------------- final linear: out = h1[T-1] @ Wlin.T + blin -------------
        # h1tps[T-1] holds transposed bf16 h1; cast to f32r for a precise matmul.
        hl32 = consts.tile([128, 256], f32r, tag="hl32")
        nc.vector.tensor_copy(hl32, h1tps[T - 1])
        outp = g0pp.tile([B, G], f32, tag="g0", name="outp")[:, 0:P_OUT]
        mm(outp, e032_sb, blinf_sb, start=True, stop=False)
        for k in range(2):
            mm(
                outp,
                hl32[:, k * 128 : (k + 1) * 128],
                wlint_sb[:, k * P_OUT : (k + 1) * P_OUT],
                start=False,
                stop=(k == 1),
            )
        out_sb = consts.tile([B, P_OUT], f32, tag="outsb")
        nc.vector.tensor_copy(out_sb, outp)
        nc.sync.dma_start(out_d[:, :], out_sb)

    nc.finalize()
    return nc


def _get_module():
    global _MODULE
    if _MODULE is None:
        _MODULE = _build_module()
    return _MODULE


def kernel(**inputs):
    global LAST_RESULTS
    from concourse.bass_utils import run_bass_kernel_spmd

    f = lambda a: np.ascontiguousarray(np.asarray(a), dtype=np.float32)
    x = f(inputs["x"])
    emb = f(inputs["emb"])
    Wih0, Whh0 = f(inputs["Wih0"]), f(inputs["Whh0"])
    bih0, bhh0 = f(inputs["bih0"]), f(inputs["bhh0"])
    Wih1, Whh1 = f(inputs["Wih1"]), f(inputs["Whh1"])
    bih1, bhh1 = f(inputs["bih1"]), f(inputs["bhh1"])
    Wlin, blin = f(inputs["Wlin"]), f(inputs["blin"])

    # Fold embedding + biases into layer-0 input weights.
    w_val = Wih0[:, 0:1]  # [G, 1]
    M0 = Wih0[:, 1 : 1 + D] @ emb.T  # [G, 7]
    b0 = (bih0 + bhh0)[:, None]  # [G, 1]
    W0aug = np.concatenate(
        [w_val, M0, b0, np.zeros((G, 128 - 9), np.float32)], axis=1
    )  # [G, 128]


    w0t_dr = np.ascontiguousarray(W0aug[_PERM].T.astype(BF))  # [128, G]

    def dr_pack_h(Wt):  # Wt: [256, G] -> [128, 2G] k-tiles side by side
        a = np.concatenate([Wt[0:128], Wt[128:256]], axis=1)
        return np.ascontiguousarray(a.astype(BF))

    whh0t_dr = dr_pack_h(np.ascontiguousarray(Whh0[_PERM].T))
    wih1t_dr = dr_pack_h(np.ascontiguousarray(Wih1[_PERM].T))
    whh1t_dr = dr_pack_h(np.ascontiguousarray(Whh1[_PERM].T))

    b1f = np.zeros((128, G), np.float32)
    b1f[0] = (bih1 + bhh1)[_PERM]
    b1f_dr = np.ascontiguousarray(b1f.astype(BF))
    e0p = np.zeros((128, 128), np.float32)
    e0p[0] = 1.0
    e0p = np.ascontiguousarray(e0p.astype(BF))
    e032 = np.zeros((128, 128), np.float32)
    e032[0] = 1.0
    wlint = np.ascontiguousarray(Wlin.T)  # [H, P_OUT]
    blinf = np.zeros((128, P_OUT), np.float32)
    blinf[0] = blin

    val = x[:, :, 0]  # [B_FULL, T]
    day = x[:, :, 1].astype(np.int32)  # [B_FULL, T]

    in_maps = []
    for c in range(N_CORES):
        sl = slice(c * B, (c + 1) * B)
        aug = np.zeros((FA, T, B), np.float32)
        aug[0] = val[sl].T
        dT = day[sl].T  # [T, B]
        for d in range(7):
            aug[1 + d] = dT == d
        aug[8] = 1.0
        in_maps.append(
            {
                "aug": np.ascontiguousarray(aug.reshape(FA, T * B).astype(BF)),
                "w0t": w0t_dr,
                "whh0t": whh0t_dr,
                "wih1t": wih1t_dr,
                "whh1t": whh1t_dr,
                "e0p": e0p,
                "b1f": b1f_dr,
                "e032": e032,
                "wlint": wlint,
                "blinf": blinf,
            }
        )

    res = run_bass_kernel_spmd(_get_module(), in_maps, core_ids=list(range(N_CORES)))
    LAST_RESULTS = res
    out = np.concatenate([r["out"] for r in res.results], axis=0)
    return np.ascontiguousarray(out, dtype=np.float32)
